# revision 1
# baseline (speedup 1.0000x reference)
"""Trainium2 Bass kernel for nn_CombineGraph (GCE-GNN LocalAggregator).

Computation (per batch b):
    h = emb_table[inputs[b]]                         # [L, D]
    e_k[i,j] = leakyrelu(sum_d h[i,d]*h[j,d]*a_k[d]) # 4 edge-type logits
    alpha = softmax_j(select-by-adj(e_k), -9e15 fill)
    out[b] = alpha @ h

Sharding: pure data-parallel over batch B=512 across 8 NeuronCores
(64 batches/core). emb_table + a-vectors replicated; no collectives.

Device algorithm per batch (transposed-softmax formulation):
  - indirect-DMA gather h' = emb_aug[idx] -> [100, 129] (col 128 == 1.0,
    pre-appended on host) in SBUF.
  - PE transpose: hT [128,100] (D on partitions).
  - scaled = hT (bcast x4) * a_pat  (one DVE broadcast tensor_tensor)
  - e = hT.T @ scaled -> PSUM [100, 400]: e[j, k*100+i] = e_k[i,j] (symmetric!)
  - m5 = (adjT bcast x5 == [1,2,3,4,0] pattern)  (one DVE is_equal; POOL is
    kept free for the indirect gathers, whose ~1.2us/op dispatch cost made
    gpsimd masks the original bottleneck)
    w[:,0:400] = m5 * e; w[:,400:500] = m5_plane4 * (-9e15)  (DVE)
  - t[j,i] = sum over 5 planes (DVE grouped reduce)
  - u = LeakyRelu(t, 0.2); pT = Exp(u)   (ACT, one table set)
    (no max-subtraction needed: |logits| are tiny; masked entries
     are -9e15 -> lrelu -> -1.8e15 -> exp -> exactly 0)
  - o = pT.T @ h' -> PSUM [100,129]; col 128 = row sums s[i]
  - out = o[:, 0:128] * (1/s)  (DVE reciprocal + tensor_scalar) -> DMA out
"""
import numpy as np

import concourse.bass as bass
import concourse.bacc as bacc
import concourse.tile as tile
from concourse import mybir
from concourse import bass_utils
from concourse.masks import make_identity

try:
    import ml_dtypes
    _BF16 = ml_dtypes.bfloat16
except ImportError:  # pragma: no cover
    import jax.numpy as jnp
    _BF16 = jnp.bfloat16

B, L, D, V = 512, 100, 128, 200000
NCORES = 8
BS = B // NCORES          # 64 batches per core
NB = 8                    # batches per adj DMA group
NEG = -9e15
NEG_SLOPE = 0.2
DA = D + 4                # h tile free size (129 used, pad to 132)


def build_nc(reps: int = 1):
    """Build + compile the per-core Bass program (SPMD, shared by all cores).

    reps>1 wraps the whole 64-batch body in a hardware loop (for timing)."""
    nc = bacc.Bacc("TRN2", target_bir_lowering=False, debug=False,
                   enable_asserts=False, num_devices=NCORES)
    f32 = mybir.dt.float32
    bf16 = mybir.dt.bfloat16
    i32 = mybir.dt.int32

    emb = nc.dram_tensor("emb", [V, D + 1], f32, kind="ExternalInput")
    idx_t = nc.dram_tensor("idx_t", [L, BS], i32, kind="ExternalInput")
    adj_t = nc.dram_tensor("adj_t", [L, BS, L], bf16, kind="ExternalInput")
    a_pat = nc.dram_tensor("a_pat", [D, 4 * L], f32, kind="ExternalInput")
    out_d = nc.dram_tensor("out", [BS, L, D], f32, kind="ExternalOutput")

    from contextlib import ExitStack
    with tile.TileContext(nc) as tc, ExitStack() as ctx:
        cp = ctx.enter_context(tc.tile_pool(name="const", bufs=1))
        adj_pool = ctx.enter_context(tc.tile_pool(name="adj", bufs=3))
        hp = ctx.enter_context(tc.tile_pool(name="hp", bufs=16))
        sb = ctx.enter_context(tc.tile_pool(name="sb", bufs=4))
        ps_hT = ctx.enter_context(tc.tile_pool(name="ps_hT", bufs=2,
                                               space="PSUM"))
        ps_e = ctx.enter_context(tc.tile_pool(name="ps_e", bufs=3,
                                              space="PSUM"))
        ps_o = ctx.enter_context(tc.tile_pool(name="ps_o", bufs=3,
                                              space="PSUM"))

        idx_sb = cp.tile([L, BS], i32)
        nc.sync.dma_start(out=idx_sb[:], in_=idx_t.ap())
        a_sb = cp.tile([D, 4 * L], f32)
        nc.sync.dma_start(out=a_sb[:], in_=a_pat.ap())
        ident = cp.tile([L, L], f32)
        make_identity(nc, ident[:])
        kpat5 = cp.tile([L, 5 * L], bf16)
        for k in range(5):
            nc.gpsimd.memset(kpat5[:, k * L:(k + 1) * L],
                             float(k + 1) if k < 4 else 0.0)
        negc = cp.tile([L, L], f32)
        nc.gpsimd.memset(negc[:], NEG)

        def body(_iv=None):
            for n in range(BS):
                grp, nn = divmod(n, NB)
                if nn == 0:
                    adj_new = adj_pool.tile([L, NB, L], bf16, tag="adj")
                    body.adj_sb = adj_new
                    nc.sync.dma_start(
                        out=adj_new[:],
                        in_=adj_t.ap()[:, grp * NB:(grp + 1) * NB, :])
                adj_sb = body.adj_sb
                adjn = adj_sb[:, nn, :]

                # gather h' (with ones column at 128)
                h = hp.tile([L, DA], f32, tag="h")
                nc.gpsimd.indirect_dma_start(
                    out=h[:, 0:D + 1], out_offset=None, in_=emb.ap(),
                    in_offset=bass.IndirectOffsetOnAxis(
                        ap=idx_sb[:, n:n + 1], axis=0))

                # hT = h.T (PE), evac to SBUF
                hT_ps = ps_hT.tile([D, L], f32, tag="hT_ps")
                nc.tensor.transpose(out=hT_ps[:], in_=h[:, 0:D],
                                    identity=ident[:])
                hT = sb.tile([D, L], f32, tag="hT")
                nc.scalar.copy(hT[:], hT_ps[:])

                # scaled[:, k] = hT * a_k
                scaled = sb.tile([D, 4 * L], f32, tag="scaled")
                nc.vector.tensor_tensor(
                    out=scaled[:].rearrange("p (k i) -> p k i", k=4),
                    in0=hT[:].unsqueeze(1).to_broadcast([D, 4, L]),
                    in1=a_sb[:].rearrange("p (k i) -> p k i", k=4),
                    op=mybir.AluOpType.mult)

                # e[j, k*100+i] = e_k (symmetric)
                e_ps = ps_e.tile([L, 4 * L], f32, tag="e_ps")
                nc.tensor.matmul(out=e_ps[:], lhsT=hT[:], rhs=scaled[:],
                                 start=True, stop=True)

                # masks + select + -inf fill
                m5 = sb.tile([L, 5 * L], bf16, tag="m5")
                nc.vector.tensor_tensor(
                    out=m5[:].rearrange("p (k i) -> p k i", k=5),
                    in0=adjn.unsqueeze(1).to_broadcast([L, 5, L]),
                    in1=kpat5[:].rearrange("p (k i) -> p k i", k=5),
                    op=mybir.AluOpType.is_equal)
                w = sb.tile([L, 5 * L], f32, tag="w")
                nc.vector.tensor_tensor(out=w[:, 0:4 * L], in0=m5[:, 0:4 * L],
                                        in1=e_ps[:],
                                        op=mybir.AluOpType.mult)
                nc.vector.tensor_tensor(out=w[:, 4 * L:5 * L],
                                        in0=m5[:, 4 * L:5 * L], in1=negc[:],
                                        op=mybir.AluOpType.mult)

                t = sb.tile([L, L], f32, tag="t")
                nc.vector.tensor_reduce(
                    out=t[:], in_=w[:].rearrange("p (k i) -> p i k", k=5),
                    axis=mybir.AxisListType.X, op=mybir.AluOpType.add)

                # pT = exp(lrelu(t))
                u = sb.tile([L, L], f32, tag="u")
                nc.scalar.activation(out=u[:], in_=t[:],
                                     func=mybir.ActivationFunctionType.Lrelu,
                                     alpha=NEG_SLOPE)
                pT = sb.tile([L, L], f32, tag="pT")
                nc.scalar.activation(out=pT[:], in_=u[:],
                                     func=mybir.ActivationFunctionType.Exp)

                # out rows + row-sums in one matmul (ones column)
                o_ps = ps_o.tile([L, D + 1], f32, tag="o_ps")
                nc.tensor.matmul(out=o_ps[:], lhsT=pT[:], rhs=h[:, 0:D + 1],
                                 start=True, stop=True)

                r = sb.tile([L, 1], f32, tag="r")
                nc.vector.reciprocal(r[:], o_ps[:, D:D + 1])
                o_sb = sb.tile([L, D], f32, tag="o_sb")
                nc.vector.tensor_scalar(out=o_sb[:], in0=o_ps[:, 0:D],
                                        scalar1=r[:, 0:1], scalar2=None,
                                        op0=mybir.AluOpType.mult)
                nc.sync.dma_start(out=out_d.ap()[n], in_=o_sb[:])

        if reps == 1:
            body()
        else:
            with tc.For_i(0, reps, 1) as iv:
                body(iv)

    nc.compile()
    return nc


_CACHED_NC = None


def _shard_inputs(inputs, adj, emb_table, a0, a1, a2, a3):
    inputs = np.asarray(inputs).astype(np.int32)
    adj = np.asarray(adj)
    emb_table = np.asarray(emb_table, dtype=np.float32)
    avecs = [np.asarray(a, dtype=np.float32) for a in (a0, a1, a2, a3)]

    emb_aug = np.concatenate(
        [emb_table, np.ones((V, 1), np.float32)], axis=1)   # [V, 129]
    a_pat = np.concatenate(
        [np.tile(a[:, None], (1, L)) for a in avecs], axis=1)  # [128, 400]

    in_maps = []
    for c in range(NCORES):
        sl = slice(c * BS, (c + 1) * BS)
        idx_c = np.ascontiguousarray(inputs[sl].T)                 # [L, BS]
        adj_c = np.ascontiguousarray(
            adj[sl].transpose(2, 0, 1)).astype(_BF16)              # [L,BS,L]
        in_maps.append(dict(emb=emb_aug, idx_t=idx_c, adj_t=adj_c,
                            a_pat=a_pat))
    return in_maps


def kernel(inputs, adj, mask_item, item, emb_table, a0, a1, a2, a3):
    """Full inputs in, full output out. mask_item/item are unused by the
    reference model's forward pass."""
    global _CACHED_NC
    if _CACHED_NC is None:
        _CACHED_NC = build_nc(reps=1)
    nc = _CACHED_NC

    in_maps = _shard_inputs(inputs, adj, emb_table, a0, a1, a2, a3)
    res = bass_utils.run_bass_kernel_spmd(nc, in_maps,
                                          core_ids=list(range(NCORES)))
    out = np.concatenate([np.asarray(res.results[c]["out"])
                          for c in range(NCORES)], axis=0)
    return out



# revision 9
# speedup vs baseline: 1.1002x; 1.1002x over previous
"""Trainium2 Bass kernel for nn_CombineGraph (GCE-GNN LocalAggregator).

Computation (per batch b):
    h = emb_table[inputs[b]]                         # [L, D]
    e_k[i,j] = leakyrelu(sum_d h[i,d]*h[j,d]*a_k[d]) # 4 edge-type logits
    alpha = softmax_j(select-by-adj(e_k), -9e15 fill)
    out[b] = alpha @ h

Sharding: pure data-parallel over batch B=512 across 8 NeuronCores
(64 batches/core). emb_table + a-vectors replicated; no collectives.

v2 changes vs baseline (237us):
  - Prelu (parametric_relu) instead of Lrelu: lives in the same ACT
    table set as Exp and Copy -> kills the per-batch LoadActFuncSet
    thrash (1.28us per swap, was ~111us total on ACT).
  - Indirect gathers batched NB=16 per dma op: SWDGE fixed overhead is
    994ns/op + 0.34ns/descriptor, so 16 batches cost ~1.6us instead of
    16us (was ~66us total on GPSIMD).
  - Edge-type masks precomputed on host (mb5: 4 one-hot planes + a
    -9e15*(adj==0) plane, bf16) -> removes is_equal + neg-fill ops.
  - bf16 hT/scaled/e-matmul (PE 1 cyc/row vs 4 for f32); w stored
    (i,k)-interleaved so the 4-plane reduce reads packed bf16 (DVE 2x).
  - Work spread across engines: gather+negadd on GPSIMD, evac/prelu/exp
    on ACT, scaled/w/reduce/recip on DVE, final scale alternates
    ACT/DVE; out DMA grouped per 16 batches (565ns SP dispatch each).

Device algorithm per batch (transposed-softmax formulation):
  - e[j, k*100+i] = e_k(i,j) (symmetric) via hT.T @ (hT*a_k bcast)
  - t[j,i] = sum_k mask_k*e + negplane; pT = Exp(Prelu(t))
  - o = pT.T @ h' (ones col -> row sums s); out = o * (1/s)
"""
import numpy as np

import concourse.bass as bass
import concourse.bacc as bacc
import concourse.tile as tile
from concourse import mybir
from concourse import bass_utils
from concourse.masks import make_identity

try:
    import ml_dtypes
    _BF16 = ml_dtypes.bfloat16
except ImportError:  # pragma: no cover
    import jax.numpy as jnp
    _BF16 = jnp.bfloat16

B, L, D, V = 512, 100, 128, 200000
NCORES = 8
BS = B // NCORES          # 64 batches per core
NB = 16                   # batches per gather / mask-DMA / out-DMA group
NEG = -9e15
NEG_SLOPE = 0.2
DA = D + 4                # h tile free size (129 used, pad to 132)


def build_nc(reps: int = 1):
    """Build + compile the per-core Bass program (SPMD, shared by all cores).

    reps>1 wraps the whole 64-batch body in a hardware loop (for timing)."""
    nc = bacc.Bacc("TRN2", target_bir_lowering=False, debug=False,
                   enable_asserts=False, num_devices=NCORES)
    f32 = mybir.dt.float32
    bf16 = mybir.dt.bfloat16
    i32 = mybir.dt.int32

    emb = nc.dram_tensor("emb", [V, D + 1], f32, kind="ExternalInput")
    idx_t = nc.dram_tensor("idx_t", [L, BS], i32, kind="ExternalInput")
    mb5_t = nc.dram_tensor("mb5_t", [L, BS, 5, L], bf16, kind="ExternalInput")
    a_pat = nc.dram_tensor("a_pat", [D, 4 * L], bf16, kind="ExternalInput")
    # [L, BS, D] (partition-major) so the grouped out-DMA reads o_grp with a
    # contiguous AP; host transposes back to [BS, L, D]
    out_d = nc.dram_tensor("out", [L, BS, D], f32, kind="ExternalOutput")

    from contextlib import ExitStack
    with tile.TileContext(nc) as tc, ExitStack() as ctx:
        cp = ctx.enter_context(tc.tile_pool(name="const", bufs=1))
        mb_pool = ctx.enter_context(tc.tile_pool(name="mb", bufs=2))
        hp = ctx.enter_context(tc.tile_pool(name="hp", bufs=2))
        op = ctx.enter_context(tc.tile_pool(name="op", bufs=2))
        sb = ctx.enter_context(tc.tile_pool(name="sb", bufs=4))
        ps_hT = ctx.enter_context(tc.tile_pool(name="ps_hT", bufs=2,
                                               space="PSUM"))
        ps_e = ctx.enter_context(tc.tile_pool(name="ps_e", bufs=3,
                                              space="PSUM"))
        ps_o = ctx.enter_context(tc.tile_pool(name="ps_o", bufs=3,
                                              space="PSUM"))

        idx_sb = cp.tile([L, BS], i32)
        nc.sync.dma_start(out=idx_sb[:], in_=idx_t.ap())
        a_sb = cp.tile([D, 4 * L], bf16)
        nc.sync.dma_start(out=a_sb[:], in_=a_pat.ap())
        ident = cp.tile([L, L], f32)
        make_identity(nc, ident[:])

        def body(_iv=None):
            for n in range(BS):
                grp, nn = divmod(n, NB)
                if nn == 0:
                    gsl = slice(grp * NB, (grp + 1) * NB)
                    mb5_new = mb_pool.tile([L, NB, 5, L], bf16, tag="mb5")
                    nc.sync.dma_start(out=mb5_new[:],
                                      in_=mb5_t.ap()[:, gsl, :, :])
                    body.mb5 = mb5_new
                    h_new = hp.tile([L, NB, DA], f32, tag="h")
                    for b in range(NB):
                        gb = grp * NB + b
                        nc.gpsimd.indirect_dma_start(
                            out=h_new[:, b, 0:D + 1], out_offset=None,
                            in_=emb.ap(),
                            in_offset=bass.IndirectOffsetOnAxis(
                                ap=idx_sb[:, gb:gb + 1], axis=0))
                    body.h_grp = h_new
                    body.o_grp = op.tile([L, NB * D], f32, tag="o")
                mb5 = body.mb5
                h_grp = body.h_grp
                o_grp = body.o_grp

                # hT = h.T (PE), evac to SBUF as bf16 (ACT)
                hT_ps = ps_hT.tile([D, L], f32, tag="hT_ps")
                nc.tensor.transpose(out=hT_ps[:], in_=h_grp[:, nn, 0:D],
                                    identity=ident[:])
                hT = sb.tile([D, L], bf16, tag="hT")
                nc.scalar.activation(out=hT[:], in_=hT_ps[:],
                                     func=mybir.ActivationFunctionType.Copy)

                # scaled[:, k] = hT * a_k  (DVE, all-bf16 all-SBUF)
                scaled = sb.tile([D, 4 * L], bf16, tag="scaled")
                nc.vector.tensor_tensor(
                    out=scaled[:].rearrange("p (k i) -> p k i", k=4),
                    in0=hT[:].unsqueeze(1).to_broadcast([D, 4, L]),
                    in1=a_sb[:].rearrange("p (k i) -> p k i", k=4),
                    op=mybir.AluOpType.mult)

                # e[j, k*100+i] = e_k (symmetric), bf16 matmul
                e_ps = ps_e.tile([L, 4 * L], f32, tag="e_ps")
                nc.tensor.matmul(out=e_ps[:], lhsT=hT[:], rhs=scaled[:],
                                 start=True, stop=True)

                # w[j, ki] = mask_k[j,i] * e_k[j,i]
                w = sb.tile([L, 4 * L], bf16, tag="w")
                nc.vector.tensor_tensor(
                    out=w[:].rearrange("p (k i) -> p k i", k=4),
                    in0=mb5[:, nn, 0:4, :],
                    in1=e_ps[:].rearrange("p (k i) -> p k i", k=4),
                    op=mybir.AluOpType.mult)
                # 4-plane sum as two packed bf16 adds (DVE 2x mode; a strided
                # tensor_reduce would run at 1x)
                t2 = sb.tile([L, 2 * L], bf16, tag="t2")
                nc.vector.tensor_tensor(out=t2[:], in0=w[:, 0:2 * L],
                                        in1=w[:, 2 * L:4 * L],
                                        op=mybir.AluOpType.add)
                t4 = sb.tile([L, L], bf16, tag="t4")
                nc.vector.tensor_tensor(out=t4[:], in0=t2[:, 0:L],
                                        in1=t2[:, L:2 * L],
                                        op=mybir.AluOpType.add)

                # t = t4 + negplane  (GPSIMD; -9e15 where adj==0), then
                # prelu as one fused op: u = max(t*0.2, t)  (GPSIMD)
                t = sb.tile([L, L], bf16, tag="t")
                nc.gpsimd.tensor_tensor(out=t[:], in0=t4[:],
                                        in1=mb5[:, nn, 4, :],
                                        op=mybir.AluOpType.add)
                u = sb.tile([L, L], bf16, tag="u")
                nc.vector.scalar_tensor_tensor(
                    out=u[:], in0=t[:], scalar=NEG_SLOPE, in1=t[:],
                    op0=mybir.AluOpType.mult, op1=mybir.AluOpType.max)

                # pT = exp(u)  (ACT; Exp+Copy share one table set)
                pT = sb.tile([L, L], f32, tag="pT")
                nc.scalar.activation(out=pT[:], in_=u[:],
                                     func=mybir.ActivationFunctionType.Exp)

                # out rows + row-sums in one matmul (ones column)
                o_ps = ps_o.tile([L, D + 1], f32, tag="o_ps")
                nc.tensor.matmul(out=o_ps[:], lhsT=pT[:],
                                 rhs=h_grp[:, nn, 0:D + 1],
                                 start=True, stop=True)

                r = sb.tile([L, 1], f32, tag="r")
                nc.vector.reciprocal(r[:], o_ps[:, D:D + 1])
                osl = o_grp[:, nn * D:(nn + 1) * D]
                if nn % 2 == 0:
                    nc.scalar.activation(
                        out=osl, in_=o_ps[:, 0:D],
                        func=mybir.ActivationFunctionType.Copy,
                        scale=r[:, 0:1])
                else:
                    nc.vector.tensor_scalar(out=osl, in0=o_ps[:, 0:D],
                                            scalar1=r[:, 0:1], scalar2=None,
                                            op0=mybir.AluOpType.mult)
                if nn == NB - 1:
                    nc.sync.dma_start(
                        out=out_d.ap()[:, grp * NB:(grp + 1) * NB, :],
                        in_=o_grp[:].rearrange("p (b d) -> p b d", b=NB))

        if reps == 1:
            body()
        else:
            with tc.For_i(0, reps, 1) as iv:
                body(iv)

    nc.compile()
    return nc


_CACHED_NC = None


def _shard_inputs(inputs, adj, emb_table, a0, a1, a2, a3):
    inputs = np.asarray(inputs).astype(np.int32)
    adj = np.asarray(adj)
    emb_table = np.asarray(emb_table, dtype=np.float32)
    avecs = [np.asarray(a, dtype=np.float32) for a in (a0, a1, a2, a3)]

    emb_aug = np.concatenate(
        [emb_table, np.ones((V, 1), np.float32)], axis=1)   # [V, 129]
    a_pat = np.concatenate(
        [np.tile(a[:, None], (1, L)) for a in avecs],
        axis=1).astype(_BF16)                               # [128, 400]

    in_maps = []
    for c in range(NCORES):
        sl = slice(c * BS, (c + 1) * BS)
        idx_c = np.ascontiguousarray(inputs[sl].T)                 # [L, BS]
        adj_c = adj[sl]                                            # [BS, i, j]
        # mb5[j, n, k, i]: planes 0-3 one-hot for edge types 1-4,
        # plane 4 = -9e15 where adj==0 (softmax mask fill)
        eq = (adj_c[:, :, :, None] ==
              np.arange(1, 5)[None, None, None, :])       # [BS, i, j, 4]
        mb5 = np.empty((L, BS, 5, L), dtype=_BF16)
        mb5[:, :, 0:4, :] = eq.transpose(2, 0, 3, 1).astype(_BF16)
        mb5[:, :, 4, :] = (NEG * (adj_c == 0)).transpose(2, 0, 1)
        in_maps.append(dict(emb=emb_aug, idx_t=idx_c,
                            mb5_t=np.ascontiguousarray(mb5), a_pat=a_pat))
    return in_maps


def kernel(inputs, adj, mask_item, item, emb_table, a0, a1, a2, a3):
    """Full inputs in, full output out. mask_item/item are unused by the
    reference model's forward pass."""
    global _CACHED_NC
    if _CACHED_NC is None:
        _CACHED_NC = build_nc(reps=1)
    nc = _CACHED_NC

    in_maps = _shard_inputs(inputs, adj, emb_table, a0, a1, a2, a3)
    res = bass_utils.run_bass_kernel_spmd(nc, in_maps,
                                          core_ids=list(range(NCORES)))
    # device layout is [L, BS, D]; transpose back to [BS, L, D]
    out = np.concatenate([np.asarray(res.results[c]["out"]).transpose(1, 0, 2)
                          for c in range(NCORES)], axis=0)
    return out
